# revision 9
# baseline (speedup 1.0000x reference)
"""GNN message-passing kernel for 8 Trainium2 NeuronCores.

Math (per reference):
  h   = relu(ef @ W1 + b1)                      [E, H]
  K   = (h @ W2 + b2).reshape(E, G, L)          per-edge [G, L] kernels
  t   = einsum('bnl,ne->bel', x, inc)           gather nodes->edges
  y   = einsum('egl,bel->beg', K, t)            per-edge matvec
  out = relu(einsum('ne,beg->bng', inc, y) + b_gc).reshape(B, N*G)

Distribution: shard the edge dim E across the 8 cores (2000 edges each).
Every stage (MLP, gather, matvec, scatter-partial) is edge-local; the
scatter partials [B, N, G] are summed on the host (the all-reduce), then
bias + relu applied.

Per-core dataflow (single NEFF, two phases):
  phase 1, per 128-edge chunk: mlp2 -> kT[l,e,g] (bf16), gather
    (xT[n,(b,l)] bf16 x inc[n,e] bf16, K=n accumulated in PSUM) ->
    tT[l,e,b] (bf16), per-edge matmul K=l -> Y psum [g,b], drain to
    Ycp[g,b,e], xbar-DMA-transpose per b -> Yfin[e,(b,g)] bf16, DMA to
    a DRAM staging buffer Y[EL, B*G].
  phase 2: scatter = incT[e,n] x Y[e,(b,g)] with PSUM accumulation over
    all 16 e-chunks (K=2000), 4 node-chunks x 8 (b,g)-chunks, DMA the
    [125, 8, 64] psum tiles straight to out[b,n,g].
"""

import numpy as np
import ml_dtypes

import concourse.bass as bass
from concourse import bacc
import concourse.mybir as mybir
import concourse.tile as tile
from concourse.bass_utils import run_bass_kernel_spmd

B, N, E, L, G, F, H = 64, 500, 16000, 64, 64, 8, 128
NCORES = 8
ELR = E // NCORES       # 2000 real edges per core
EL = 2048               # padded to a multiple of EC; pad edges have zero
                        # incidence columns so they contribute nothing
EC = 128                # edge chunk
NCH = EL // EC          # 16 chunks
NP = 125                # nodes per n-chunk (500 = 4*125)
NQ = 4                  # n-chunks
BG = B * G              # 4096
F32 = mybir.dt.float32
BF16 = mybir.dt.bfloat16
RELU = mybir.ActivationFunctionType.Relu
IDENT = mybir.ActivationFunctionType.Identity

_CACHE = {}
last_results = None     # BassKernelResults of the most recent run (for test.py)


def _build():
    nc = bacc.Bacc("TRN2", target_bir_lowering=False)
    xT_d = nc.declare_dram_parameter("xT", [N, B * L], BF16, isOutput=False)
    inc_d = nc.declare_dram_parameter("inc", [N, EL], BF16, isOutput=False)
    incT_d = nc.declare_dram_parameter("incT", [EL, N], BF16, isOutput=False)
    efT_d = nc.declare_dram_parameter("efT", [F, EL], BF16, isOutput=False)
    W1_d = nc.declare_dram_parameter("W1", [F, H], BF16, isOutput=False)
    b1_d = nc.declare_dram_parameter("b1", [H, 1], F32, isOutput=False)
    W2_d = nc.declare_dram_parameter("W2", [H, G * L], BF16, isOutput=False)
    b2T_d = nc.declare_dram_parameter("b2T", [H, G * L // H], F32, isOutput=False)
    out_d = nc.declare_dram_parameter("out", [B, N, G], F32, isOutput=True)
    y_d = nc.dram_tensor("Ystage", [EL, BG], BF16)

    with tile.TileContext(nc) as tc, tc.tile_pool(name="const", bufs=1) as cpool:
        with tc.tile_pool(name="h_ps", bufs=2, space="PSUM") as hps:
            # ---- persistent tiles ----
            xT_sb = cpool.tile([NP, NQ, B * L], BF16)       # 32KB/part
            nc.sync.dma_start(
                out=xT_sb[:, :, :],
                in_=xT_d[:, :].rearrange("(q n) c -> n q c", q=NQ),
            )
            W1_sb = cpool.tile([F, H], BF16)
            nc.sync.dma_start(out=W1_sb[:, :], in_=W1_d[:, :])
            b1_sb = cpool.tile([H, 1], F32)
            nc.sync.dma_start(out=b1_sb[:, :], in_=b1_d[:, :])
            W2_sb = cpool.tile([H, G * L], BF16)            # 8KB/part
            nc.sync.dma_start(out=W2_sb[:, :], in_=W2_d[:, :])
            b2T_sb = cpool.tile([H, G * L // H], F32)
            nc.sync.dma_start(out=b2T_sb[:, :], in_=b2T_d[:, :])
            efT_sb = cpool.tile([F, EL], BF16)
            nc.sync.dma_start(out=efT_sb[:, :], in_=efT_d[:, :])
            hT_sb = cpool.tile([H, EL], BF16)               # 4KB/part

            # ---- mlp1: hT = relu(W1.T @ efT + b1), all edges upfront ----
            for c in range(4):
                ph = hps.tile([H, 512], F32)
                nc.tensor.matmul(
                    ph[:, :], lhsT=W1_sb[:, :],
                    rhs=efT_sb[:, c * 512:(c + 1) * 512],
                    start=True, stop=True,
                )
                nc.scalar.activation(
                    hT_sb[:, c * 512:(c + 1) * 512], ph[:, :], RELU,
                    bias=b1_sb[:, 0:1],
                )

        # ---- phase 1 ----
        with (
            tc.tile_pool(name="stream", bufs=2) as spool,
            tc.tile_pool(name="kt", bufs=2) as ktpool,
            tc.tile_pool(name="tt", bufs=2) as ttpool,
            tc.tile_pool(name="ycp", bufs=2) as ycppool,
            tc.tile_pool(name="yfin", bufs=2) as yfpool,
            tc.tile_pool(name="mlp2_ps", bufs=2, space="PSUM") as mps,
            tc.tile_pool(name="gat_ps", bufs=2, space="PSUM") as gps,
            tc.tile_pool(name="mv_ps", bufs=2, space="PSUM") as vps,
        ):
            for ch in range(NCH):
                e0 = ch * EC
                # mlp2: kernels for this chunk -> kT[l, e, g] bf16 (+b2)
                kT = ktpool.tile([L, EC, G], BF16, tag="kt")
                for mc in range(32):
                    pm = mps.tile([H, EC], F32, tag="m2")
                    nc.tensor.matmul(
                        pm[:, :], lhsT=W2_sb[:, mc * H:(mc + 1) * H],
                        rhs=hT_sb[:, e0:e0 + EC], start=True, stop=True,
                    )
                    for par in (0, 1):
                        src = pm[par * 64:(par + 1) * 64, :]
                        dst = kT[:, :, 2 * mc + par]
                        bias = b2T_sb[par * 64:(par + 1) * 64, mc:mc + 1]
                        if (2 * mc + par) % 3 == 0:
                            nc.scalar.activation(dst, src, IDENT, bias=bias)
                        else:
                            nc.vector.tensor_scalar_add(dst, src, bias)

                # gather: tT[l, e, b] bf16
                inc_t = spool.tile([NP, NQ, EC], BF16, tag="inc")
                nc.sync.dma_start(
                    out=inc_t[:, :, :],
                    in_=inc_d[:, e0:e0 + EC].rearrange("(q n) e -> n q e", q=NQ),
                )
                tT = ttpool.tile([L, EC, B], BF16, tag="tt")
                for bp in range(B // 2):
                    pg = gps.tile([2 * L, EC], F32, tag="g")
                    for q in range(NQ):
                        nc.tensor.matmul(
                            pg[:, :],
                            lhsT=xT_sb[:, q, bp * 128:(bp + 1) * 128],
                            rhs=inc_t[:, q, :],
                            start=(q == 0), stop=(q == NQ - 1),
                        )
                    for par in (0, 1):
                        src = pg[par * 64:(par + 1) * 64, :]
                        dst = tT[:, :, 2 * bp + par]
                        if (2 * bp + par) % 3 == 0:
                            nc.scalar.copy(dst, src)
                        else:
                            nc.vector.tensor_copy(dst, src)

                # per-edge matvec: Y[g, b] per edge -> Ycp[g, b, e]
                ycp = ycppool.tile([G, B, EC], BF16, tag="ycp")
                for j in range(EC // 8):
                    pv = vps.tile([G, 8, B], F32, tag="mv")
                    for k in range(8):
                        er = j * 8 + k
                        nc.tensor.matmul(
                            pv[:, k, :], lhsT=kT[:, er, :], rhs=tT[:, er, :],
                            start=True, stop=True,
                        )
                    dst = ycp[:, :, j * 8:(j + 1) * 8].transpose([0, 2, 1])
                    if j % 3 == 0:
                        nc.scalar.copy(dst, pv[:, :, :])
                    else:
                        nc.vector.tensor_copy(dst, pv[:, :, :])

                # transpose: Yfin[e, b, g] bf16 via xbar DMA per b
                yfin = yfpool.tile([EC, B, G], BF16, tag="yf")
                for b in range(B):
                    nc.scalar.dma_start(
                        out=yfin[:, b, :], in_=ycp[:, b, :], transpose=True,
                    )
                nc.sync.dma_start(
                    out=y_d[e0:e0 + EC, :],
                    in_=yfin[:, :, :],
                )

        # ---- phase 2: scatter with PSUM accumulation over all edges ----
        with (
            tc.tile_pool(name="p2c", bufs=1) as p2c,
            tc.tile_pool(name="p2rhs", bufs=3) as p2r,
            tc.tile_pool(name="acc_ps", bufs=8, space="PSUM") as aps,
        ):
            incT_sb = p2c.tile([EC, NCH, N], BF16)          # 16KB/part
            nc.sync.dma_start(
                out=incT_sb[:, :, :],
                in_=incT_d[:, :].rearrange("(c e) n -> e c n", c=NCH),
            )
            for nj in range(BG // 512):
                paccs = [aps.tile([NP, 8, G], F32, tag="acc", name=f"acc{nj}_{m}")
                         for m in range(NQ)]
                for ec in range(NCH):
                    rt = p2r.tile([EC, 512], BF16, tag="rhs")
                    nc.sync.dma_start(
                        out=rt[:, :],
                        in_=y_d[ec * EC:(ec + 1) * EC, nj * 512:(nj + 1) * 512],
                    )
                    for m in range(NQ):
                        nc.tensor.matmul(
                            paccs[m][:, :, :],
                            lhsT=incT_sb[:, ec, m * NP:(m + 1) * NP],
                            rhs=rt[:, :],
                            start=(ec == 0), stop=(ec == NCH - 1),
                        )
                for m in range(NQ):
                    ot = p2r.tile([NP, 8, G], F32, tag="ostage", name=f"ost{nj}_{m}")
                    if m % 2 == 0:
                        nc.vector.tensor_copy(ot[:, :, :], paccs[m][:, :, :])
                    else:
                        nc.scalar.copy(ot[:, :, :], paccs[m][:, :, :])
                    nc.sync.dma_start(
                        out=out_d[nj * 8:(nj + 1) * 8,
                                  m * NP:(m + 1) * NP, :].transpose([1, 0, 2]),
                        in_=ot[:, :, :],
                    )
    nc.compile()
    return nc


def kernel(x, incidence, ef, W1, b1, W2, b2, b_gc):
    global last_results
    x = np.asarray(x, dtype=np.float32)
    incidence = np.asarray(incidence, dtype=np.float32)
    ef = np.asarray(ef, dtype=np.float32)
    W1 = np.asarray(W1, dtype=np.float32)
    b1 = np.asarray(b1, dtype=np.float32)
    W2 = np.asarray(W2, dtype=np.float32)
    b2 = np.asarray(b2, dtype=np.float32)
    b_gc = np.asarray(b_gc, dtype=np.float32)

    if "nc" not in _CACHE:
        _CACHE["nc"] = _build()
    nc = _CACHE["nc"]

    bf = ml_dtypes.bfloat16
    xT = np.ascontiguousarray(
        x.transpose(1, 0, 2).reshape(N, B * L)).astype(bf)
    inc_bf = incidence.astype(bf)
    incT_bf = np.ascontiguousarray(incidence.T).astype(bf)
    efT = np.ascontiguousarray(ef.T).astype(bf)
    b1c = np.ascontiguousarray(b1.reshape(H, 1))
    W2_bf = W2.astype(bf)
    b2T = np.ascontiguousarray(b2.reshape(G * L // H, H).T)

    pad = EL - ELR
    in_maps = []
    for c in range(NCORES):
        es = slice(c * ELR, (c + 1) * ELR)
        in_maps.append({
            "xT": xT,
            "inc": np.ascontiguousarray(
                np.pad(inc_bf[:, es], ((0, 0), (0, pad)))),
            "incT": np.ascontiguousarray(
                np.pad(incT_bf[es, :], ((0, pad), (0, 0)))),
            "efT": np.ascontiguousarray(
                np.pad(efT[:, es], ((0, 0), (0, pad)))),
            "W1": W1.astype(bf), "b1": b1c, "W2": W2_bf, "b2T": b2T,
        })

    import os
    trace = bool(int(os.environ.get("KERNEL_TRACE", "0")))
    last_results = run_bass_kernel_spmd(
        nc, in_maps, list(range(NCORES)), trace=trace)
    partial = np.zeros((B, N, G), np.float32)
    for r in last_results.results:
        partial += r["out"]
    out = np.maximum(partial + b_gc.reshape(1, 1, G), 0.0)
    return out.reshape(B, N * G).astype(np.float32)
